# revision 27
# baseline (speedup 1.0000x reference)
"""Bass/Trainium2 kernel for nn_Attention (B=4, N=2048, IN=256, HID=1024,
D=1024, OUT=256, H=8 heads), SPMD over 8 NeuronCores.

Sharding: core c handles batch b = c//2 and head-group g = c%2 (4 heads,
512 of the 1024 inner features).  Layer-1 of each QKV MLP is recomputed on
both cores of a batch (cheap); the output projection is computed per
head-group and the two partial products are summed on the host (plus bias).

Mask compaction: ~half the tokens are masked out (key mask) and masked
queries only output the bias row.  The host applies ONE permutation
(valid tokens first) to q, k and v inputs, so the kernel runs on
NP = ceil(max_valid/128)*128 tokens instead of N=2048.  Using the same
permutation on both sides keeps the no-self-attention mask a true
diagonal.  Masked/padded key rows get an additive -30000 before exp.

Per-core dataflow (laid out so no on-chip transposes are ever needed):
  xT (256,NP) -> L1 feature-major h=(1024,NP) tanh -> L2:
     qT,kT feature-major bf16 (512,NP) = 4 head tiles [128,NP]
     v token-major bf16 (NP,512) (bias added via rank-1 matmul)
  attention per (head, q-chunk): S^T tiles [128 k-tok, qw] = kT_t.T @ qT;
     key mask enters as the per-partition bias of the Exp activation; the
     diagonal is one [128,128] additive DVE op; denominators via a
     [128,128] all-ones stationary matmul over the bf16 exp-accumulator
     (sum arrives broadcast across partitions); 1/s = exp(-ln s) on the
     Scalar engine; y^T accumulates in PSUM and is copied out immediately.
  proj: out^T = Wp_g^T @ (y^T * 1/s) in bf16.
"""

import numpy as np

B, N, IN_DIM, HID, D, OUT_DIM, H = 4, 2048, 256, 1024, 1024, 256, 8
NCORES = 8
HG = 2                 # head groups (cores per batch)
DG = D // HG           # 512 features per group
HEADS_G = H // HG      # 4 heads per core
Dh = D // H            # 128
NEG = -30000.0         # additive mask value (exp underflows to 0)

_CACHE = {}


def _chunks(total, size):
    out = []
    o = 0
    while o < total:
        s = min(size, total - o)
        out.append((o, s))
        o += s
    return out


def _build_nc(NP):
    import concourse.mybir as mybir
    import concourse.tile as tile
    from concourse import bacc
    from contextlib import ExitStack

    dt = mybir.dt
    f32 = dt.float32
    f32r = dt.float32r
    bf16 = dt.bfloat16
    AF = mybir.ActivationFunctionType
    ALU = mybir.AluOpType

    # Keep Ln and Exp in ONE activation-table set: blank out the funcs of
    # every exp/ln-capable set except exp_and_others (tanh+exp, phase A)
    # and natural_log_exp_and_others (exp+ln, phase B), so the table-load
    # pass never picks a set that would thrash between Ln and Exp.
    # Positions (= act_func_set_id) are preserved.
    if not getattr(bacc, "_act_tables_patched", False):
        from concourse import hw_specs as _hw
        _orig_get = _hw.get_activation_tables

        def _patched(arch):
            tables = dict(_orig_get(arch))
            AFT = mybir.ActivationFunctionType
            keep = {"exp_and_others", "natural_log_exp_and_others"}
            for name in tables:
                if name in keep:
                    continue
                fns = tables[name]
                if AFT.Exp in fns or AFT.Ln in fns:
                    tables[name] = set()
            return tables

        _patched.__wrapped__ = _orig_get
        bacc.get_activation_tables = _patched
        bacc._act_tables_patched = True

    nc = bacc.Bacc("TRN2", target_bir_lowering=False, debug=False)

    # ---- DRAM I/O ----
    xqT = nc.dram_tensor("xqT", [IN_DIM, NP], f32r, kind="ExternalInput")
    xkT = nc.dram_tensor("xkT", [IN_DIM, NP], f32r, kind="ExternalInput")
    xvT = nc.dram_tensor("xvT", [IN_DIM, NP], f32r, kind="ExternalInput")
    wq1 = nc.dram_tensor("wq1", [IN_DIM, HID], f32r, kind="ExternalInput")
    wk1 = nc.dram_tensor("wk1", [IN_DIM, HID], f32r, kind="ExternalInput")
    wv1 = nc.dram_tensor("wv1", [IN_DIM, HID], f32r, kind="ExternalInput")
    bq1 = nc.dram_tensor("bq1", [128, HID // 128], f32, kind="ExternalInput")
    bk1 = nc.dram_tensor("bk1", [128, HID // 128], f32, kind="ExternalInput")
    bv1 = nc.dram_tensor("bv1", [128, HID // 128], f32, kind="ExternalInput")
    wq2 = nc.dram_tensor("wq2", [HID, DG], bf16, kind="ExternalInput")
    wk2 = nc.dram_tensor("wk2", [HID, DG], bf16, kind="ExternalInput")
    wv2 = nc.dram_tensor("wv2", [HID, DG], bf16, kind="ExternalInput")
    bq2 = nc.dram_tensor("bq2", [128, DG // 128], f32, kind="ExternalInput")
    bk2 = nc.dram_tensor("bk2", [128, DG // 128], f32, kind="ExternalInput")
    bv2r = nc.dram_tensor("bv2r", [128, DG], f32r, kind="ExternalInput")
    onesc = nc.dram_tensor("onesc", [128, 128], bf16, kind="ExternalInput")
    e0Td = nc.dram_tensor("e0Td", [128, 128], f32r, kind="ExternalInput")
    wpb = nc.dram_tensor("wpb", [DG, OUT_DIM], bf16, kind="ExternalInput")
    kmadd = nc.dram_tensor("kmadd", [128, NP // 128], f32,
                           kind="ExternalInput")
    dneg = nc.dram_tensor("dneg", [128, 128], f32, kind="ExternalInput")
    outT = nc.dram_tensor("outT", [OUT_DIM, NP], f32, kind="ExternalOutput")

    KT1 = IN_DIM // 128          # 2  k-tiles in layer 1
    KT2 = HID // 128             # 8  k-tiles in layer 2
    MT1 = HID // 128             # 8  m-tiles in layer 1
    NTOK = NP // 128             # key-token tiles
    QCH = _chunks(NP, 1024)      # attention q-chunks
    THC = _chunks(NP, 1024)      # MLP token chunks

    with tile.TileContext(nc) as tc, ExitStack() as ctx:
        # pools (PSUM: ps 3x2 banks + psy 1x2 banks = 8 banks)
        ps = ctx.enter_context(tc.tile_pool(name="ps", bufs=3, space="PSUM"))
        psy = ctx.enter_context(tc.tile_pool(name="psy", bufs=1, space="PSUM"))
        singles = ctx.enter_context(tc.tile_pool(name="singles", bufs=1))
        xt_pool = ctx.enter_context(tc.tile_pool(name="xt", bufs=4))
        w1_pool = ctx.enter_context(tc.tile_pool(name="w1", bufs=4))
        w2_pool = ctx.enter_context(tc.tile_pool(name="w2", bufs=8))
        h_pool = ctx.enter_context(tc.tile_pool(name="h", bufs=8))
        qk_pool = ctx.enter_context(tc.tile_pool(name="qk", bufs=8))
        v_pool = ctx.enter_context(
            tc.tile_pool(name="v", bufs=(NTOK + 3) // 4))
        pt_pool = ctx.enter_context(tc.tile_pool(name="pt", bufs=8))
        sacc_pool = ctx.enter_context(tc.tile_pool(name="sacc", bufs=3))
        ysc_pool = ctx.enter_context(tc.tile_pool(name="ysc", bufs=6))
        rb_pool = ctx.enter_context(tc.tile_pool(name="rb", bufs=3))
        y2s_pool = ctx.enter_context(tc.tile_pool(name="y2s", bufs=4))
        out_pool = ctx.enter_context(tc.tile_pool(name="out", bufs=2))

        # constants
        ones128 = singles.tile([128, 128], bf16, tag="ones128")
        nc.sync.dma_start(out=ones128, in_=onesc[:, :])
        e0T = singles.tile([128, 128], f32r, tag="e0T")
        nc.sync.dma_start(out=e0T, in_=e0Td[:, :])
        km_sb = singles.tile([128, NP // 128], f32, tag="km")
        nc.sync.dma_start(out=km_sb, in_=kmadd[:, :])
        dneg_sb = singles.tile([128, 128], f32, tag="dneg")
        nc.sync.dma_start(out=dneg_sb, in_=dneg[:, :])
        bv2_sb = singles.tile([128, DG], f32r, tag="bv2")
        nc.sync.dma_start(out=bv2_sb, in_=bv2r[:, :])
        wp_sb = singles.tile([128, HEADS_G, OUT_DIM], bf16, tag="wp")
        nc.sync.dma_start(
            out=wp_sb, in_=wpb.rearrange("(h p) o -> p h o", p=128)
        )
        b1_sb = {}
        b2_sb = {}
        for t, (b1d, b2d) in {
            "q": (bq1, bq2), "k": (bk1, bk2), "v": (bv1, None)
        }.items():
            b1_sb[t] = singles.tile(
                [128, HID // 128], f32, tag=f"b1{t}", name=f"b1{t}")
            nc.sync.dma_start(out=b1_sb[t], in_=b1d[:, :])
            if b2d is not None:
                b2_sb[t] = singles.tile(
                    [128, DG // 128], f32, tag=f"b2{t}", name=f"b2{t}")
                nc.sync.dma_start(out=b2_sb[t], in_=b2d[:, :])

        # persistent activations
        qT = [qk_pool.tile([128, NP], bf16, tag="qk", name=f"qT{i}")
              for i in range(HEADS_G)]
        kT = [qk_pool.tile([128, NP], bf16, tag="qk", name=f"kT{i}")
              for i in range(HEADS_G)]
        v_sb = [v_pool.tile([128, 4 * DG], bf16, tag="v", name=f"v{i}")
                for i in range((NTOK + 3) // 4)]

        # ---------------- phase A: the three MLPs (v first: all Tanh
        # activations retire before attention's Exp stream begins) --------
        for t, xd, w1d, w2d in (
            ("v", xvT, wv1, wv2), ("k", xkT, wk1, wk2), ("q", xqT, wq1, wq2)
        ):
            w1_sb = []
            for k in range(KT1):
                w1t = w1_pool.tile([128, HID], f32r, tag="w1")
                nc.sync.dma_start(out=w1t, in_=w1d[k * 128:(k + 1) * 128, :])
                w1_sb.append(w1t)
            w2_sb = []
            for k in range(KT2):
                w2t = w2_pool.tile([128, DG], bf16, tag="w2")
                nc.sync.dma_start(out=w2t, in_=w2d[k * 128:(k + 1) * 128, :])
                w2_sb.append(w2t)

            for t0, tsz in THC:
                tok_sl = slice(t0, t0 + tsz)
                xts = []
                for k in range(KT1):
                    xt = xt_pool.tile([128, 1024], f32r, tag="xt")
                    nc.sync.dma_start(
                        out=xt[:, :tsz], in_=xd[k * 128:(k + 1) * 128, tok_sl]
                    )
                    xts.append(xt)
                # layer 1 (feature-major)
                h_sb = []
                for m in range(MT1):
                    p1 = ps.tile([128, 1024], f32, tag="ps")
                    for k in range(KT1):
                        for q0, qs in _chunks(tsz, 512):
                            nc.tensor.matmul(
                                p1[:, q0:q0 + qs],
                                w1_sb[k][:, m * 128:(m + 1) * 128],
                                xts[k][:, q0:q0 + qs],
                                start=(k == 0), stop=(k == KT1 - 1),
                            )
                    ht = h_pool.tile([128, 1024], bf16, tag="h")
                    nc.scalar.activation(
                        out=ht[:, :tsz], in_=p1[:, :tsz], func=AF.Tanh,
                        bias=b1_sb[t][:, m:m + 1], scale=1.0,
                    )
                    h_sb.append(ht)
                # layer 2
                if t in ("q", "k"):
                    dst = qT if t == "q" else kT
                    for m in range(DG // 128):       # head tiles
                        p2 = ps.tile([128, 1024], f32, tag="ps")
                        for k in range(KT2):
                            for q0, qs in _chunks(tsz, 512):
                                nc.tensor.matmul(
                                    p2[:, q0:q0 + qs],
                                    w2_sb[k][:, m * 128:(m + 1) * 128],
                                    h_sb[k][:, q0:q0 + qs],
                                    start=(k == 0), stop=(k == KT2 - 1),
                                )
                        nc.vector.tensor_scalar_add(
                            out=dst[m][:, tok_sl], in0=p2[:, :tsz],
                            scalar1=b2_sb[t][:, m:m + 1],
                        )
                else:
                    # v: token-major [tok, feat], bias via rank-1 matmul
                    ntiles = tsz // 128
                    for tp in range(0, ntiles, 2):   # pairs of token tiles
                        npair = min(2, ntiles - tp)
                        pv = ps.tile([128, 1024], f32, tag="ps")
                        for tt in range(npair):
                            sl = slice(tt * 512, (tt + 1) * 512)
                            for k in range(KT2):
                                nc.tensor.matmul(
                                    pv[:, sl],
                                    h_sb[k][:, (tp + tt) * 128:
                                            (tp + tt + 1) * 128],
                                    w2_sb[k][:, :],
                                    start=(k == 0), stop=False,
                                )
                            nc.tensor.matmul(
                                pv[:, sl], e0T[:, :], bv2_sb[:, :],
                                start=False, stop=True,
                            )
                        tok0 = t0 // 128 + tp
                        nc.vector.tensor_copy(
                            out=v_sb[tok0 // 4][
                                :, (tok0 % 4) * 512:
                                (tok0 % 4 + npair) * 512],
                            in_=pv[:, :npair * 512],
                        )

        # ---------------- phase B: attention + projection ----------------
        for q0, qw in QCH:
            ysc_tiles = []
            for hd in range(HEADS_G):
                y2 = psy.tile([128, 1024], f32, tag="y2")
                sacc = sacc_pool.tile([128, 1024], bf16, tag="sacc")
                for kt in range(NTOK):
                    st = ps.tile([128, 1024], f32, tag="ps")
                    for c0, cs in _chunks(qw, 512):
                        nc.tensor.matmul(
                            st[:, c0:c0 + cs],
                            kT[hd][:, kt * 128:(kt + 1) * 128],
                            qT[hd][:, q0 + c0:q0 + c0 + cs],
                            start=True, stop=True,
                        )
                    off = kt * 128 - q0
                    if 0 <= off <= qw - 128:
                        nc.vector.tensor_tensor(
                            st[:, off:off + 128], st[:, off:off + 128],
                            dneg_sb, ALU.add,
                        )
                    pt = pt_pool.tile([128, 1024], bf16, tag="pt")
                    nc.scalar.activation(
                        out=pt[:, :qw], in_=st[:, :qw], func=AF.Exp,
                        bias=km_sb[:, kt:kt + 1], scale=1.0,
                    )
                    if kt == 0:
                        nc.vector.tensor_copy(
                            out=sacc[:, :qw], in_=pt[:, :qw])
                    else:
                        nc.vector.tensor_tensor(
                            sacc[:, :qw], sacc[:, :qw], pt[:, :qw], ALU.add)
                    vt = v_sb[kt // 4][
                        :, (kt % 4) * 512 + hd * 128:
                        (kt % 4) * 512 + (hd + 1) * 128]
                    for c0, cs in _chunks(qw, 512):
                        nc.tensor.matmul(
                            y2[:, c0:c0 + cs], vt,
                            pt[:, c0:c0 + cs],
                            start=(kt == 0), stop=(kt == NTOK - 1),
                        )
                # free the y2 PSUM slot immediately
                y2s = y2s_pool.tile([128, 1024], f32, tag="y2s")
                nc.vector.tensor_copy(out=y2s[:, :qw], in_=y2[:, :qw])
                # denominators: all-ones stationary matmul -> sums broadcast
                aux = ps.tile([128, 1024], f32, tag="ps")
                for c0, cs in _chunks(qw, 512):
                    nc.tensor.matmul(
                        aux[:, c0:c0 + cs], ones128[:, :],
                        sacc[:, c0:c0 + cs],
                        start=True, stop=True,
                    )
                # 1/s = exp(-ln(s)) on the Scalar engine
                lns = rb_pool.tile([128, 1024], f32, tag="rb")
                nc.scalar.activation(
                    out=lns[:, :qw], in_=aux[:, :qw], func=AF.Ln, scale=1.0)
                rb2 = rb_pool.tile([128, 1024], f32, tag="rb")
                nc.scalar.activation(
                    out=rb2[:, :qw], in_=lns[:, :qw], func=AF.Exp,
                    scale=-1.0)
                ysc = ysc_pool.tile([128, 1024], bf16, tag="ysc")
                nc.vector.tensor_tensor(
                    ysc[:, :qw], y2s[:, :qw], rb2[:, :qw], ALU.mult)
                ysc_tiles.append(ysc)
            # projection for this q-chunk
            for od in range(OUT_DIM // 128):
                pp = ps.tile([128, 1024], f32, tag="ps")
                for c0, cs in _chunks(qw, 512):
                    for hd in range(HEADS_G):
                        nc.tensor.matmul(
                            pp[:, c0:c0 + cs],
                            wp_sb[:, hd, od * 128:(od + 1) * 128],
                            ysc_tiles[hd][:, c0:c0 + cs],
                            start=(hd == 0), stop=(hd == HEADS_G - 1),
                        )
                ot = out_pool.tile([128, 1024], f32, tag="out")
                nc.vector.tensor_copy(out=ot[:, :qw], in_=pp[:, :qw])
                nc.sync.dma_start(
                    out=outT[od * 128:(od + 1) * 128, q0:q0 + qw],
                    in_=ot[:, :qw],
                )

    nc.compile()
    return nc


def _row0_pad(row, nrows):
    out = np.zeros((nrows, row.shape[0]), np.float32)
    out[0] = row
    return out


def _e0t():
    out = np.zeros((128, 128), np.float32)
    out[0, :] = 1.0
    return out


def _perm_np(mask_b):
    """Valid-first stable permutation and valid count for one batch."""
    maskf = mask_b.astype(np.float32)
    perm = np.argsort(1.0 - maskf, kind="stable")
    nv = int(maskf.sum())
    return perm, nv


def _pad_tokens(x, NP):
    """x: (N, F) -> (NP, F) zero-padded/truncated token dim."""
    out = np.zeros((NP, x.shape[1]), np.float32)
    n = min(NP, x.shape[0])
    out[:n] = x[:n]
    return out


def _prep_core_inputs(inputs, b, g, NP):
    import ml_dtypes

    f32 = np.float32
    bf = ml_dtypes.bfloat16
    sl = slice(g * DG, (g + 1) * DG)
    scale = float(Dh) ** -0.5
    perm, nv = _perm_np(inputs["mask"][b, :, 0])
    km = np.full(NP, NEG, f32)
    km[:nv] = 0.0
    dn = np.zeros((128, 128), f32)
    np.fill_diagonal(dn, NEG)

    def ptok(x):   # permute tokens valid-first, pad to NP
        return _pad_tokens(x[perm].astype(f32), NP)

    return {
        "xqT": np.ascontiguousarray(ptok(inputs["query"][b]).T),
        "xkT": np.ascontiguousarray(ptok(inputs["key"][b]).T),
        "xvT": np.ascontiguousarray(ptok(inputs["value"][b]).T),
        "wq1": np.ascontiguousarray(inputs["Wq1"].astype(f32)),
        "wk1": np.ascontiguousarray(inputs["Wk1"].astype(f32)),
        "wv1": np.ascontiguousarray(inputs["Wv1"].astype(f32)),
        "bq1": np.ascontiguousarray(
            inputs["bq1"].astype(f32).reshape(HID // 128, 128).T),
        "bk1": np.ascontiguousarray(
            inputs["bk1"].astype(f32).reshape(HID // 128, 128).T),
        "bv1": np.ascontiguousarray(
            inputs["bv1"].astype(f32).reshape(HID // 128, 128).T),
        "wq2": np.ascontiguousarray(
            (inputs["Wq2"][:, sl].astype(f32) * scale).astype(bf)),
        "wk2": np.ascontiguousarray(inputs["Wk2"][:, sl].astype(bf)),
        "wv2": np.ascontiguousarray(inputs["Wv2"][:, sl].astype(bf)),
        "bq2": np.ascontiguousarray(
            (inputs["bq2"][sl].astype(f32) * scale).reshape(DG // 128, 128).T),
        "bk2": np.ascontiguousarray(
            inputs["bk2"][sl].astype(f32).reshape(DG // 128, 128).T),
        "bv2r": _row0_pad(inputs["bv2"][sl].astype(f32), 128),
        "onesc": np.ones((128, 128), bf),
        "e0Td": _e0t(),
        "wpb": np.ascontiguousarray(inputs["Wp"][sl, :].astype(bf)),
        "kmadd": np.ascontiguousarray(km.reshape(NP // 128, 128).T),
        "dneg": dn,
    }


def kernel(**inputs):
    import sys
    if "/opt/trn_rl_repo" not in sys.path:
        sys.path.insert(0, "/opt/trn_rl_repo")
    from concourse.bass_utils import run_bass_kernel_spmd

    inputs = {k: np.asarray(v) for k, v in inputs.items()}

    nv_max = int(inputs["mask"][:, :, 0].sum(axis=1).max())
    NP = ((nv_max + 127) // 128) * 128

    if _CACHE.get("NP") != NP:
        _CACHE["nc"] = _build_nc(NP)
        _CACHE["NP"] = NP
    nc = _CACHE["nc"]

    in_maps = [
        _prep_core_inputs(inputs, c // HG, c % HG, NP) for c in range(NCORES)
    ]

    res = run_bass_kernel_spmd(nc, in_maps, core_ids=list(range(NCORES)))
    results = res.results

    bp = inputs["bp"].astype(np.float32)
    out = np.empty((B, N, OUT_DIM), np.float32)
    for b in range(B):
        acc = results[b * HG]["outT"].astype(np.float32)
        for g in range(1, HG):
            acc = acc + results[b * HG + g]["outT"].astype(np.float32)
        perm, nv = _perm_np(inputs["mask"][b, :, 0])
        out[b] = bp[None, :]
        out[b, perm[:nv]] = acc.T[:nv] + bp[None, :]
    return out
